# revision 1
# baseline (speedup 1.0000x reference)
"""Circular-relative-bias multi-head attention on 8 Trainium2 NeuronCores.

Sharding (Megatron MHA): 16 heads -> 2 heads per core. Each core computes
q/k/v projections for its 128 channels (2 heads x 64), full attention for
its heads over both batches, and a row-sharded output projection producing
a full-shape partial; the host sums the 8 partials and adds bo.

Layout strategy: the host pre-packs every input into the exact SBUF layout
the kernel wants, so all DMAs are linear:
  - xt      [1024, 4096]        x transposed (d-major)       f32
  - wq/wk/wv[128, 8, 128]       [k-part, d-tile, ch]         f32 (wq,bq pre-scaled 1/8)
  - wo      [128, 1024]         [ch, d]                      f32
  - bq/bk/bv[128, 1]            per-channel bias             f32
  - ebias   [128, 2, 28, 512]   exp(rel bias) tiles, by      bf16
                                [i, head, diag-class, j]

Attention works on transposed scores P^T [sk, sq] so softmax sums come free
from the attn@V matmul via a ones-column in the stationary operand:
  head0 lhsT = v_store[:, t, 0:65]  = [v0 | ones]      -> accumA rows 0-63 data, 64 sums
  head1 lhsT = v_store[:, t, 1:129] = [.. | ones | v1] -> accumB rows 64-127 data, 63 sums
This puts head1's outputs in partitions 64-127 directly (lane-aligned with
its slot in the [128, tok] attnout tile) at zero extra matmul cost.

exp(s + b) = exp(s) * exp(b): the circular bias enters as a precomputed
elementwise bf16 multiplier; [128, 512] score tiles along the same
(512*qb - 128*kt) diagonal share one of 28 classes per head.

No max-subtraction in softmax: scores ~ N(0,1) + 0.02-bias, |s| < ~7 over
4M samples, exp stays well inside f32 range.
"""

import math

import numpy as np
import ml_dtypes

B = 2
S = 2048
D = 1024
H = 16
HD = 64
PERIOD = 4096
NCORES = 8
CH = 128          # channels per core = 2 heads * 64
TOK = B * S       # 4096
DT = D // 128     # 8 k-tiles for the d contraction
TB = 256          # token block for projections
NTB = TOK // TB   # 16
SQ = 512          # sq block in attention
NQB = S // SQ     # 4 per batch
SK = 128          # sk tile
NKT = S // SK     # 16 per batch
NCLS = NQB * 4 + NKT - 4  # 28 diagonal classes: 4*qb - kt in [-15, 12]

_CACHE = {}


def _build_nc():
    import contextlib

    import concourse.tile as tile
    from concourse import bacc, mybir
    from concourse.masks import make_identity

    f32 = mybir.dt.float32
    f32r = mybir.dt.float32r
    bf16 = mybir.dt.bfloat16

    nc = bacc.Bacc("TRN2")
    xt = nc.dram_tensor("xt", [D, TOK], bf16, kind="ExternalInput")
    wq = nc.dram_tensor("wq", [128, DT, CH], bf16, kind="ExternalInput")
    wk = nc.dram_tensor("wk", [128, DT, CH], bf16, kind="ExternalInput")
    wv = nc.dram_tensor("wv", [128, DT, CH], bf16, kind="ExternalInput")
    wo = nc.dram_tensor("wo", [CH, D], bf16, kind="ExternalInput")
    bq = nc.dram_tensor("bq", [CH, 1], f32, kind="ExternalInput")
    bk = nc.dram_tensor("bk", [CH, 1], f32, kind="ExternalInput")
    bv = nc.dram_tensor("bv", [CH, 1], f32, kind="ExternalInput")
    ebias = nc.dram_tensor("ebias", [128, 2, NCLS, SQ], bf16, kind="ExternalInput")
    o_part = nc.dram_tensor("o_part", [TOK, D], f32, kind="ExternalOutput")

    with tile.TileContext(nc) as tc, contextlib.ExitStack() as ctx:
        singles = ctx.enter_context(tc.tile_pool(name="singles", bufs=1))
        xt_pool = ctx.enter_context(tc.tile_pool(name="xt", bufs=2))
        vt_pool = ctx.enter_context(tc.tile_pool(name="vt", bufs=2))
        ep_pool = ctx.enter_context(tc.tile_pool(name="ep", bufs=4))
        nrm_pool = ctx.enter_context(tc.tile_pool(name="nrm", bufs=2))
        ao_pool = ctx.enter_context(tc.tile_pool(name="ao", bufs=2))
        out_pool = ctx.enter_context(tc.tile_pool(name="out", bufs=2))
        mm_ps = ctx.enter_context(tc.tile_pool(name="mmps", bufs=3, space="PSUM"))
        acc_ps = ctx.enter_context(tc.tile_pool(name="accps", bufs=2, space="PSUM"))

        ident = singles.tile([128, 128], bf16)
        make_identity(nc, ident)

        w_sb = {}
        b_sb = {}
        for name, w_h, b_h in (("q", wq, bq), ("k", wk, bk), ("v", wv, bv)):
            w_sb[name] = singles.tile([128, DT, CH], bf16, tag=f"w{name}", name=f"w{name}_sb")
            nc.sync.dma_start(out=w_sb[name], in_=w_h[:, :, :])
            b_sb[name] = singles.tile([CH, 1], f32, tag=f"b{name}", name=f"b{name}_sb")
            nc.sync.dma_start(out=b_sb[name], in_=b_h[:, :])
        wo_sb = singles.tile([CH, D], bf16, tag="wo")
        nc.sync.dma_start(out=wo_sb, in_=wo[:, :])
        eb_sb = singles.tile([128, 2, NCLS, SQ], bf16, tag="eb")
        nc.sync.dma_start(out=eb_sb, in_=ebias[:, :, :, :])

        # q^T / k^T stores [ch, tok]; v_store [tok-part, tok-tile, 129]
        qT = singles.tile([CH, TOK], bf16, tag="qT")
        kT = singles.tile([CH, TOK], bf16, tag="kT")
        # v_store cols: [v0: 0..63 | ones: 64 | zeros: 65..95 | v1: 96..159]
        # head0 lhsT = [:, t, 0:65]   -> acc rows 0-63 data, 64 sums
        # head1 lhsT = [:, t, 32:160] -> acc row 32 sums, rows 64-127 data
        # (rows 0-31/33-63 of accB get junk from v0 cols — never read)
        v_store = singles.tile([128, TOK // 128, 160], bf16, tag="vst")
        nc.vector.memset(v_store[:, :, 64:65], 1.0)
        nc.vector.memset(v_store[:, :, 65:96], 0.0)

        xt_v = xt.rearrange("(dt k) t -> k dt t", k=128)

        # ---- projections ----
        for tb in range(NTB):
            ts = tb * TB
            xt_sb = xt_pool.tile([128, DT, TB], bf16, tag="xt")
            nc.sync.dma_start(out=xt_sb, in_=xt_v[:, :, ts : ts + TB])
            for name, store in (("q", qT), ("k", kT), ("v", None)):
                ps = mm_ps.tile([CH, TB], f32, tag="mm")
                for dt in range(DT):
                    nc.tensor.matmul(
                        ps,
                        w_sb[name][:, dt],
                        xt_sb[:, dt],
                        start=(dt == 0),
                        stop=(dt == DT - 1),
                    )
                if store is not None:
                    nc.vector.tensor_scalar_add(
                        out=store[:, ts : ts + TB], in0=ps, scalar1=b_sb[name]
                    )
                else:
                    vt_sb = vt_pool.tile([CH, TB], bf16, tag="vt")
                    nc.vector.tensor_scalar_add(
                        out=vt_sb, in0=ps, scalar1=b_sb["v"]
                    )
                    # transpose v^T -> v rows, split heads into v_store
                    for j in range(TB // 128):
                        t_idx = (ts + j * 128) // 128
                        vps = mm_ps.tile([128, 128], bf16, tag="mm")
                        nc.tensor.transpose(
                            vps, vt_sb[:, j * 128 : (j + 1) * 128], ident
                        )
                        # cols 0:64 -> head0 slot, cols 64:128 -> head1 slot
                        nc.vector.tensor_copy(
                            v_store[:, t_idx, 0:64], vps[:, 0:64]
                        )
                        nc.vector.tensor_copy(
                            v_store[:, t_idx, 96:160], vps[:, 64:128]
                        )

        # ---- attention + output projection, per batch ----
        for b in range(B):
            base = b * S
            attnout = ao_pool.tile([CH, S], bf16, tag="ao")
            for qb in range(NQB):
                q0 = base + qb * SQ
                accA = acc_ps.tile([128, SQ], f32, tag="acc")
                accB = acc_ps.tile([128, SQ], f32, tag="acc")
                for kt in range(NKT):
                    k0 = base + kt * SK
                    cls = 4 * qb - kt + (NKT - 1)
                    t_idx = k0 // 128
                    # both heads' score tiles in one 2-bank psum tile; the two
                    # QK matmuls are adjacent and use disjoint row groups
                    # (base partitions 0 / 64) so they overlap in the array
                    ps = mm_ps.tile([128, 2, SQ], f32, tag="mm")
                    for hh in (0, 1):
                        nc.tensor.matmul(
                            ps[:, hh, :],
                            kT[hh * 64 : (hh + 1) * 64, k0 : k0 + SK],
                            qT[hh * 64 : (hh + 1) * 64, q0 : q0 + SQ],
                            start=True,
                            stop=True,
                        )
                    e_sb = ep_pool.tile([128, 2, SQ], bf16, tag="e")
                    nc.scalar.activation(
                        out=e_sb, in_=ps, func=mybir.ActivationFunctionType.Exp
                    )
                    p_sb = ep_pool.tile([128, 2, SQ], bf16, tag="p")
                    nc.vector.tensor_mul(p_sb, e_sb, eb_sb[:, :, cls, :])
                    for hh, acc in ((0, accA), (1, accB)):
                        lo, width = (0, 65) if hh == 0 else (32, 128)
                        nc.tensor.matmul(
                            acc[0:width, :],
                            v_store[:, t_idx, lo : lo + width],
                            p_sb[:, hh, :],
                            start=(kt == 0),
                            stop=(kt == NKT - 1),
                        )
                # normalize: head0 sums at accA row 64, head1 sums at accB row 63
                for hh, acc, srow in ((0, accA, 64), (1, accB, 32)):
                    r = nrm_pool.tile([1, SQ], f32, tag="r")
                    nc.vector.reciprocal(r, acc[srow : srow + 1, :])
                    rb = nrm_pool.tile([64, SQ], f32, tag="rb")
                    nc.gpsimd.partition_broadcast(rb, r)
                    dlo = 0 if hh == 0 else 64
                    nc.vector.tensor_mul(
                        attnout[dlo : dlo + 64, qb * SQ : (qb + 1) * SQ],
                        acc[dlo : dlo + 64, :],
                        rb,
                    )
            # ---- output projection for this batch ----
            for ts in range(S // 128):
                o_sb = out_pool.tile([128, D], f32, tag="o")
                for half in range(2):
                    ps = mm_ps.tile([128, 512], f32, tag="mm")
                    nc.tensor.matmul(
                        ps,
                        attnout[:, ts * 128 : (ts + 1) * 128],
                        wo_sb[:, half * 512 : (half + 1) * 512],
                        start=True,
                        stop=True,
                    )
                    nc.vector.tensor_copy(o_sb[:, half * 512 : (half + 1) * 512], ps)
                nc.sync.dma_start(
                    out=o_part[base + ts * 128 : base + (ts + 1) * 128, :], in_=o_sb
                )
    nc.compile()
    return nc


def _prep_inputs(x, wq, bq, wk, bk, wv, bv, wo, bo, rel_bias):
    """Host-side pack into per-core in_maps (all linear-DMA layouts)."""
    x = np.asarray(x, dtype=np.float32)
    rel_bias = np.asarray(rel_bias, dtype=np.float32)
    scale = 1.0 / math.sqrt(HD)

    xt = np.ascontiguousarray(x.reshape(TOK, D).T).astype(ml_dtypes.bfloat16)  # [D, TOK]

    # exp-bias tiles: ebt[i, hh, cls, j] = exp(rel_bias[(c0 - i + j) % PERIOD, h])
    ii = np.arange(128)[:, None]
    jj = np.arange(SQ)[None, :]
    cls_idx = np.empty((NCLS, 128, SQ), dtype=np.int64)
    for cls in range(NCLS):
        c0 = 128 * (cls - (NKT - 1))
        cls_idx[cls] = (c0 - ii + jj) % PERIOD

    in_maps = []
    for c in range(NCORES):
        sl = slice(c * CH, (c + 1) * CH)
        wq_c = (np.asarray(wq, np.float32)[:, sl] * scale).reshape(DT, 128, CH)
        wk_c = np.asarray(wk, np.float32)[:, sl].reshape(DT, 128, CH)
        wv_c = np.asarray(wv, np.float32)[:, sl].reshape(DT, 128, CH)
        eb = np.empty((128, 2, NCLS, SQ), dtype=ml_dtypes.bfloat16)
        for hh in range(2):
            h = 2 * c + hh
            # [NCLS, 128, SQ] -> [128, NCLS, SQ]
            eb[:, hh] = np.exp(rel_bias[cls_idx, h]).transpose(1, 0, 2)
        in_maps.append(
            {
                "xt": xt,
                "wq": np.ascontiguousarray(wq_c.transpose(1, 0, 2)).astype(ml_dtypes.bfloat16),
                "wk": np.ascontiguousarray(wk_c.transpose(1, 0, 2)).astype(ml_dtypes.bfloat16),
                "wv": np.ascontiguousarray(wv_c.transpose(1, 0, 2)).astype(ml_dtypes.bfloat16),
                "wo": np.ascontiguousarray(np.asarray(wo, np.float32)[sl, :]).astype(ml_dtypes.bfloat16),
                "bq": (np.asarray(bq, np.float32)[sl] * scale).reshape(CH, 1),
                "bk": np.asarray(bk, np.float32)[sl].reshape(CH, 1),
                "bv": np.asarray(bv, np.float32)[sl].reshape(CH, 1),
                "ebias": eb,
            }
        )
    return in_maps


def kernel(x, wq, bq, wk, bk, wv, bv, wo, bo, rel_bias, _trace=False):
    from concourse import bass_utils

    if "nc" not in _CACHE:
        _CACHE["nc"] = _build_nc()
    nc = _CACHE["nc"]

    in_maps = _prep_inputs(x, wq, bq, wk, bk, wv, bv, wo, bo, rel_bias)
    res = bass_utils.run_bass_kernel_spmd(
        nc, in_maps, core_ids=list(range(NCORES)), trace=_trace
    )
    _CACHE["last_result"] = res

    acc = np.zeros((TOK, D), dtype=np.float64)
    for r in res.results:
        acc += r["o_part"].astype(np.float64)
    acc += np.asarray(bo, np.float64)[None, :]
    return acc.reshape(B, S, D).astype(np.float32)



# revision 15
# speedup vs baseline: 1.0242x; 1.0242x over previous
"""Circular-relative-bias multi-head attention on 8 Trainium2 NeuronCores.

Sharding (Megatron MHA): 16 heads -> 2 heads per core. Each core computes
q/k/v projections for its 128 channels (2 heads x 64), full attention for
its heads over both batches, and a row-sharded output projection producing
a full-shape partial; the host sums the 8 partials and adds bo.

Pipeline layout (v2):
  - proj(b0) -> attn(b0) [proj(b1) + oproj(b0) interleaved] -> attn(b1)
    [oproj(b1) interleaved]; oproj matmul+DMA units are deferred and
    drained one per kt2 iteration of the NEXT qb so the tensor queue
    never stalls on an output DMA.
  - exp(rel_bias) enters as a per-head shifted slab [128, 3968]:
    slab[p, u] = exp(rel_bias[(u - p - 1920) % 4096]); the tile for
    (qb, kt) is the slice [*, off:off+512] with off = 512*qb - 128*kt
    + 1920.  1.94 MB DMA instead of a 7.3 MB precomputed tile table.
  - scores^T [sk, sq]; softmax sums come free from the attn@V matmul via
    ones-columns in v_store (head0 sums at accA row 64, head1 at accB
    row 32); reciprocal via reciprocal_approx_fast (~51 ULP, plenty).
  - attn@V pairs two consecutive key tiles [128, 2, 512]; with fp8
    (USE_FP8_ATTN) the pair contracts in one DoubleRow matmul at 2x rate.
"""

import math

import numpy as np
import ml_dtypes

B = 2
S = 2048
D = 1024
H = 16
HD = 64
PERIOD = 4096
NCORES = 8
CH = 128          # channels per core = 2 heads * 64
TOK = B * S       # 4096
DT = D // 128     # 8 k-tiles for the d contraction
TBP = 512         # token block for projections
NTB = S // TBP    # 4 per batch
SQ = 512          # sq block in attention
NQB = S // SQ     # 4 per batch
SK = 128          # sk tile
NKT = S // SK     # 16 per batch
NK2 = NKT // 2    # 8 kt-pairs
U0 = 1920         # slab shift so off >= 0
SLABW = 3 * SQ + (NKT - 1) * SK + U0 - 1920 + SQ  # 3968
USE_FP8_ATTN = False  # S2 flag: fp8 p/v + DoubleRow attn@V
EXP_BIAS = 0.0        # subtract from scores pre-exp (cancels in softmax)
DEFER_OPROJ = True    # spread oproj units into the next qb's kt2 loop
INTERLEAVE_PROJ = True  # emit b1 projections under b0 attention

_CACHE = {}


def _build_nc():
    import contextlib

    import concourse.tile as tile
    from concourse import bacc, mybir
    from concourse.masks import make_identity

    f32 = mybir.dt.float32
    bf16 = mybir.dt.bfloat16
    fp8 = mybir.dt.float8e4
    p_dt = fp8 if USE_FP8_ATTN else bf16

    nc = bacc.Bacc("TRN2")
    xt = nc.dram_tensor("xt", [D, TOK], bf16, kind="ExternalInput")
    wq = nc.dram_tensor("wq", [128, DT, CH], bf16, kind="ExternalInput")
    wk = nc.dram_tensor("wk", [128, DT, CH], bf16, kind="ExternalInput")
    wv = nc.dram_tensor("wv", [128, DT, CH], bf16, kind="ExternalInput")
    wo = nc.dram_tensor("wo", [CH, D], bf16, kind="ExternalInput")
    bq = nc.dram_tensor("bq", [CH, 1], f32, kind="ExternalInput")
    bk = nc.dram_tensor("bk", [CH, 1], f32, kind="ExternalInput")
    bv = nc.dram_tensor("bv", [CH, 1], f32, kind="ExternalInput")
    slab = nc.dram_tensor("slab", [128, 2, SLABW], bf16, kind="ExternalInput")
    o_part = nc.dram_tensor("o_part", [TOK, D], f32, kind="ExternalOutput")

    with tile.TileContext(nc) as tc, contextlib.ExitStack() as ctx:
        singles = ctx.enter_context(tc.tile_pool(name="singles", bufs=1))
        xt_pool = ctx.enter_context(tc.tile_pool(name="xtp", bufs=2))
        vt_pool = ctx.enter_context(tc.tile_pool(name="vtp", bufs=2))
        e_pool = ctx.enter_context(tc.tile_pool(name="ep", bufs=3))
        p_pool = ctx.enter_context(tc.tile_pool(name="pp", bufs=3))
        nrm_pool = ctx.enter_context(tc.tile_pool(name="nrm", bufs=2))
        ao_pool = ctx.enter_context(tc.tile_pool(name="ao", bufs=2))
        out_pool = ctx.enter_context(tc.tile_pool(name="outp", bufs=2))
        ps = ctx.enter_context(tc.tile_pool(name="ps", bufs=2, space="PSUM"))

        ident = singles.tile([128, 128], bf16)
        make_identity(nc, ident)

        w_sb = {}
        b_sb = {}
        for name, w_h, b_h in (("q", wq, bq), ("k", wk, bk), ("v", wv, bv)):
            w_sb[name] = singles.tile([128, DT, CH], bf16, tag=f"w{name}", name=f"w{name}_sb")
            nc.sync.dma_start(out=w_sb[name], in_=w_h[:, :, :])
            b_sb[name] = singles.tile([CH, 1], f32, tag=f"b{name}", name=f"b{name}_sb")
            nc.sync.dma_start(out=b_sb[name], in_=b_h[:, :])
        wo_sb = singles.tile([CH, D], bf16, tag="wo")
        nc.sync.dma_start(out=wo_sb, in_=wo[:, :])
        slab_sb = singles.tile([128, 2, SLABW], bf16, tag="slab")

        # q^T / k^T stores [ch, tok]; v_store [tok-part, tok-tile, 160]
        qT = singles.tile([CH, TOK], bf16, tag="qT")
        kT = singles.tile([CH, TOK], bf16, tag="kT")
        # v_store cols: [v0: 0..63 | ones: 64 | zeros: 65..95 | v1: 96..159]
        # head0 lhsT = [:, t, 0:65]   -> acc rows 0-63 data, 64 sums
        # head1 lhsT = [:, t, 32:160] -> acc row 32 sums, rows 64-127 data
        v_store = singles.tile([128, TOK // 128, 160], p_dt, tag="vst")
        nc.vector.memset(v_store[:, :, 64:65], 1.0)
        nc.vector.memset(v_store[:, :, 65:96], 0.0)

        xt_v = xt.rearrange("(dt k) t -> k dt t", k=128)

        def proj_block(b, tb, emit_slab=False):
            ts = b * S + tb * TBP
            xt_sb = xt_pool.tile([128, DT, TBP], bf16, tag="xt", name=f"xt_{b}_{tb}")
            nc.sync.dma_start(out=xt_sb, in_=xt_v[:, :, ts : ts + TBP])
            if emit_slab:
                nc.sync.dma_start(out=slab_sb, in_=slab[:, :, :])
            for name in ("q", "k", "v"):
                pp = ps.tile([CH, TBP], f32, tag="pj", name=f"pj_{b}_{tb}_{name}")
                for dt in range(DT):
                    nc.tensor.matmul(
                        pp,
                        w_sb[name][:, dt],
                        xt_sb[:, dt],
                        start=(dt == 0),
                        stop=(dt == DT - 1),
                    )
                if name == "q":
                    nc.vector.tensor_scalar_add(
                        out=qT[:, ts : ts + TBP], in0=pp, scalar1=b_sb["q"]
                    )
                elif name == "k":
                    nc.vector.tensor_scalar_add(
                        out=kT[:, ts : ts + TBP], in0=pp, scalar1=b_sb["k"]
                    )
                else:
                    vt_sb = vt_pool.tile([CH, TBP], bf16, tag="vt", name=f"vt_{b}_{tb}")
                    nc.vector.tensor_scalar_add(out=vt_sb, in0=pp, scalar1=b_sb["v"])
                    for j in range(TBP // 128):
                        t_idx = (ts + j * 128) // 128
                        vps = ps.tile([128, 128], bf16, tag="pj", name=f"vps_{b}_{tb}_{j}")
                        nc.tensor.transpose(
                            vps, vt_sb[:, j * 128 : (j + 1) * 128], ident
                        )
                        nc.vector.tensor_copy(v_store[:, t_idx, 0:64], vps[:, 0:64])
                        nc.vector.tensor_copy(v_store[:, t_idx, 96:160], vps[:, 64:128])

        pend = []  # deferred small tensor-work units (oproj mm+dma)

        def drain_one():
            if pend:
                pend.pop(0)()

        def oproj_defer(b, qb, ao):
            base = b * S
            for i in range(SQ // 128):
                row = base + qb * SQ + i * 128
                for half in range(2):
                    def unit(i=i, half=half, row=row, ao=ao, b=b, qb=qb):
                        op = ps.tile(
                            [128, SQ], f32, tag="qk", name=f"op_{b}_{qb}_{i}_{half}"
                        )
                        nc.tensor.matmul(
                            op,
                            ao[:, i * 128 : (i + 1) * 128],
                            wo_sb[:, half * 512 : (half + 1) * 512],
                            start=True,
                            stop=True,
                        )
                        o_sb = out_pool.tile(
                            [128, SQ], f32, tag="o", name=f"o_{b}_{qb}_{i}_{half}"
                        )
                        nc.vector.tensor_copy(o_sb, op)
                        nc.sync.dma_start(
                            out=o_part[row : row + 128, half * 512 : (half + 1) * 512],
                            in_=o_sb,
                        )
                    if DEFER_OPROJ:
                        pend.append(unit)
                    else:
                        unit()

        def attn_v(b, qb, t2, p_sb, accA, accB):
            base = b * S
            for hh, acc, lo, w in ((0, accA, 0, 65), (1, accB, 32, 128)):
                t_idx = (base + 2 * t2 * SK) // 128
                if USE_FP8_ATTN:
                    nc.tensor.matmul(
                        acc[0:w, :],
                        v_store[:, t_idx : t_idx + 2, lo : lo + w],
                        p_sb[hh][:, :, :],
                        start=(t2 == 0),
                        stop=(t2 == NK2 - 1),
                        perf_mode=mybir.MatmulPerfMode.DoubleRow,
                    )
                else:
                    for i in (0, 1):
                        nc.tensor.matmul(
                            acc[0:w, :],
                            v_store[:, t_idx + i, lo : lo + w],
                            p_sb[hh][:, i, :],
                            start=(t2 == 0 and i == 0),
                            stop=(t2 == NK2 - 1 and i == 1),
                        )

        def attention(b):
            base = b * S
            for qb in range(NQB):
                q0 = base + qb * SQ
                accA = ps.tile([128, SQ], f32, tag="acc", name=f"accA_{b}_{qb}")
                accB = ps.tile([128, SQ], f32, tag="acc", name=f"accB_{b}_{qb}")
                prev = None
                for t2 in range(NK2):
                    # ---- QK for both heads, kt pair (paired row groups) ----
                    psh = [
                        ps.tile([128, 2, SQ], f32, tag="qk", name=f"qk_{b}_{qb}_{t2}_{hh}")
                        for hh in (0, 1)
                    ]
                    for i in (0, 1):
                        k0 = base + (2 * t2 + i) * SK
                        for hh in (0, 1):
                            nc.tensor.matmul(
                                psh[hh][:, i, :],
                                kT[hh * 64 : (hh + 1) * 64, k0 : k0 + SK],
                                qT[hh * 64 : (hh + 1) * 64, q0 : q0 + SQ],
                                start=True,
                                stop=True,
                            )
                    drain_one()  # spread deferred oproj work into the loop
                    if prev is not None:
                        attn_v(b, qb, prev, prev_p, accA, accB)
                    # ---- exp ----
                    e_sb = [
                        e_pool.tile([128, 2, SQ], bf16, tag="e", name=f"e_{b}_{qb}_{t2}_{hh}")
                        for hh in (0, 1)
                    ]
                    for hh in (0, 1):
                        nc.scalar.activation(
                            out=e_sb[hh],
                            in_=psh[hh],
                            func=mybir.ActivationFunctionType.Exp,
                            bias=EXP_BIAS,
                        )
                    # ---- p = e * exp(bias) slab slices (3 DVE / 1 Pool) ----
                    p_sb = [
                        p_pool.tile([128, 2, SQ], p_dt, tag="p", name=f"p_{b}_{qb}_{t2}_{hh}")
                        for hh in (0, 1)
                    ]
                    for hh in (0, 1):
                        for i in (0, 1):
                            off = qb * SQ - (2 * t2 + i) * SK + U0
                            eng = nc.gpsimd if hh == 1 else nc.vector
                            eng.tensor_mul(
                                p_sb[hh][:, i, :],
                                e_sb[hh][:, i, :],
                                slab_sb[:, hh, off : off + SQ],
                            )
                    prev = t2
                    prev_p = p_sb
                attn_v(b, qb, prev, prev_p, accA, accB)
                # ---- normalize: 1/sum, broadcast, scale into attnout ----
                ao = ao_pool.tile([CH, SQ], bf16, tag="ao", name=f"ao_{b}_{qb}")
                for hh, acc, srow, dlo in ((0, accA, 64, 0), (1, accB, 32, 64)):
                    r = nrm_pool.tile([1, SQ], f32, tag="r", name=f"r_{b}_{qb}_{hh}")
                    rs = nrm_pool.tile([1, SQ], f32, tag="rs", name=f"rs_{b}_{qb}_{hh}")
                    nc.vector.tensor_copy(rs, acc[srow : srow + 1, :])
                    nc.vector.reciprocal_approx_fast(out=r, in_=rs)
                    rb = nrm_pool.tile([64, SQ], f32, tag="rb", name=f"rb_{b}_{qb}_{hh}")
                    nc.gpsimd.partition_broadcast(rb, r)
                    nc.vector.tensor_mul(ao[dlo : dlo + 64, :], acc[dlo : dlo + 64, :], rb)
                # ---- interleave next-batch projections under this attention
                if b == 0 and INTERLEAVE_PROJ:
                    proj_block(1, qb)
                oproj_defer(b, qb, ao)

        # ---- batch 0 projections (prologue) ----
        for tb in range(NTB):
            proj_block(0, tb, emit_slab=(tb == 0))
        if not INTERLEAVE_PROJ:
            for tb in range(NTB):
                proj_block(1, tb)
        attention(0)
        attention(1)
        while pend:
            drain_one()
    nc.compile()
    return nc


def _prep_inputs(x, wq, bq, wk, bk, wv, bv, wo, bo, rel_bias):
    """Host-side pack into per-core in_maps (all linear-DMA layouts)."""
    x = np.asarray(x, dtype=np.float32)
    rel_bias = np.asarray(rel_bias, dtype=np.float32)
    scale = 1.0 / math.sqrt(HD)

    xt = np.ascontiguousarray(x.reshape(TOK, D).T).astype(ml_dtypes.bfloat16)  # [D, TOK]

    # slab[p, u] = exp(rel_bias[(u - p - U0) % PERIOD, h])
    uu = np.arange(SLABW)[None, :]
    pp = np.arange(128)[:, None]
    slab_idx = (uu - pp - U0) % PERIOD  # [128, SLABW]

    in_maps = []
    for c in range(NCORES):
        sl = slice(c * CH, (c + 1) * CH)
        wq_c = (np.asarray(wq, np.float32)[:, sl] * scale).reshape(DT, 128, CH)
        wk_c = np.asarray(wk, np.float32)[:, sl].reshape(DT, 128, CH)
        wv_c = np.asarray(wv, np.float32)[:, sl].reshape(DT, 128, CH)
        slab_c = np.empty((128, 2, SLABW), dtype=ml_dtypes.bfloat16)
        for hh in range(2):
            h = 2 * c + hh
            slab_c[:, hh, :] = np.exp(rel_bias[slab_idx, h])
        in_maps.append(
            {
                "xt": xt,
                "wq": np.ascontiguousarray(wq_c.transpose(1, 0, 2)).astype(ml_dtypes.bfloat16),
                "wk": np.ascontiguousarray(wk_c.transpose(1, 0, 2)).astype(ml_dtypes.bfloat16),
                "wv": np.ascontiguousarray(wv_c.transpose(1, 0, 2)).astype(ml_dtypes.bfloat16),
                "wo": np.ascontiguousarray(np.asarray(wo, np.float32)[sl, :]).astype(ml_dtypes.bfloat16),
                "bq": (np.asarray(bq, np.float32)[sl] * scale).reshape(CH, 1),
                "bk": np.asarray(bk, np.float32)[sl].reshape(CH, 1),
                "bv": np.asarray(bv, np.float32)[sl].reshape(CH, 1),
                "slab": slab_c,
            }
        )
    return in_maps


def kernel(x, wq, bq, wk, bk, wv, bv, wo, bo, rel_bias, _trace=False):
    from concourse import bass_utils

    if "nc" not in _CACHE:
        _CACHE["nc"] = _build_nc()
    nc = _CACHE["nc"]

    in_maps = _prep_inputs(x, wq, bq, wk, bk, wv, bv, wo, bo, rel_bias)
    res = bass_utils.run_bass_kernel_spmd(
        nc, in_maps, core_ids=list(range(NCORES)), trace=_trace
    )
    _CACHE["last_result"] = res

    acc = np.zeros((TOK, D), dtype=np.float64)
    for r in res.results:
        acc += r["o_part"].astype(np.float64)
    acc += np.asarray(bo, np.float64)[None, :]
    return acc.reshape(B, S, D).astype(np.float32)


# revision 24
# speedup vs baseline: 1.0757x; 1.0503x over previous
"""Circular-relative-bias multi-head attention on 8 Trainium2 NeuronCores.

Sharding (Megatron MHA): 16 heads -> 2 heads per core. Each core computes
q/k/v projections for its 128 channels (2 heads x 64), full attention for
its heads over both batches, and a row-sharded output projection producing
a full-shape partial; the host sums the 8 partials and adds bo.

Pipeline layout (v2):
  - proj(b0) -> attn(b0) [proj(b1) + oproj(b0) interleaved] -> attn(b1)
    [oproj(b1) interleaved]; oproj matmul+DMA units are deferred and
    drained one per kt2 iteration of the NEXT qb so the tensor queue
    never stalls on an output DMA.
  - exp(rel_bias) enters as a per-head shifted slab [128, 3968]:
    slab[p, u] = exp(rel_bias[(u - p - 1920) % 4096]); the tile for
    (qb, kt) is the slice [*, off:off+512] with off = 512*qb - 128*kt
    + 1920.  1.94 MB DMA instead of a 7.3 MB precomputed tile table.
  - scores^T [sk, sq]; softmax sums come free from the attn@V matmul via
    ones-columns in v_store (head0 sums at accA row 64, head1 at accB
    row 32); reciprocal via reciprocal_approx_fast (~51 ULP, plenty).
  - attn@V pairs two consecutive key tiles [128, 2, 512]; with fp8
    (USE_FP8_ATTN) the pair contracts in one DoubleRow matmul at 2x rate.
"""

import math

import numpy as np
import ml_dtypes

B = 2
S = 2048
D = 1024
H = 16
HD = 64
PERIOD = 4096
NCORES = 8
CH = 128          # channels per core = 2 heads * 64
TOK = B * S       # 4096
DT = D // 128     # 8 k-tiles for the d contraction
TBP = 512         # token block for projections
NTB = S // TBP    # 4 per batch
SQ = 512          # sq block in attention
NQB = S // SQ     # 4 per batch
SK = 128          # sk tile
NKT = S // SK     # 16 per batch
NK2 = NKT // 2    # 8 kt-pairs
U0 = 1920         # slab shift so off >= 0
SLABW = 3 * SQ + (NKT - 1) * SK + U0 - 1920 + SQ  # 3968
USE_FP8_ATTN = False  # fp8 attn@V fails the 2e-2 gate (each of p/v alone ~2.2e-2)
EXP_BIAS = -2.0       # subtract from scores pre-exp (cancels in softmax)
DEFER_OPROJ = True    # spread oproj units into the next qb's kt2 loop
INTERLEAVE_PROJ = True  # emit b1 projections under b0 attention

_CACHE = {}


def _build_nc():
    import contextlib

    import concourse.tile as tile
    from concourse import bacc, mybir
    from concourse.masks import make_identity

    f32 = mybir.dt.float32
    bf16 = mybir.dt.bfloat16
    fp8 = mybir.dt.float8e4
    p_dt = fp8 if USE_FP8_ATTN else bf16

    nc = bacc.Bacc("TRN2")
    xt = nc.dram_tensor("xt", [D, TOK], bf16, kind="ExternalInput")
    wq = nc.dram_tensor("wq", [128, DT, CH], bf16, kind="ExternalInput")
    wk = nc.dram_tensor("wk", [128, DT, CH], bf16, kind="ExternalInput")
    wv = nc.dram_tensor("wv", [128, DT, CH], bf16, kind="ExternalInput")
    wo = nc.dram_tensor("wo", [CH, D], bf16, kind="ExternalInput")
    bq = nc.dram_tensor("bq", [CH, 1], f32, kind="ExternalInput")
    bk = nc.dram_tensor("bk", [CH, 1], f32, kind="ExternalInput")
    bv = nc.dram_tensor("bv", [CH, 1], f32, kind="ExternalInput")
    slab = nc.dram_tensor("slab", [128, 2, SLABW], bf16, kind="ExternalInput")
    o_part = nc.dram_tensor("o_part", [TOK, D], f32, kind="ExternalOutput")

    with tile.TileContext(nc) as tc, contextlib.ExitStack() as ctx:
        singles = ctx.enter_context(tc.tile_pool(name="singles", bufs=1))
        xt_pool = ctx.enter_context(tc.tile_pool(name="xtp", bufs=2))
        vt_pool = ctx.enter_context(tc.tile_pool(name="vtp", bufs=2))
        p_pool = ctx.enter_context(tc.tile_pool(name="pp", bufs=3))
        nrm_pool = ctx.enter_context(tc.tile_pool(name="nrm", bufs=2))
        ao_pool = ctx.enter_context(tc.tile_pool(name="ao", bufs=2))
        out_pool = ctx.enter_context(tc.tile_pool(name="outp", bufs=2))
        ps = ctx.enter_context(tc.tile_pool(name="ps", bufs=2, space="PSUM"))

        ident = singles.tile([128, 128], bf16)
        make_identity(nc, ident)

        w_sb = {}
        b_sb = {}
        for name, w_h, b_h in (("q", wq, bq), ("k", wk, bk), ("v", wv, bv)):
            w_sb[name] = singles.tile([128, DT, CH], bf16, tag=f"w{name}", name=f"w{name}_sb")
            nc.sync.dma_start(out=w_sb[name], in_=w_h[:, :, :])
            b_sb[name] = singles.tile([CH, 1], f32, tag=f"b{name}", name=f"b{name}_sb")
            nc.sync.dma_start(out=b_sb[name], in_=b_h[:, :])
        wo_sb = singles.tile([CH, D], bf16, tag="wo")
        nc.sync.dma_start(out=wo_sb, in_=wo[:, :])
        slab_sb = singles.tile([128, 2, SLABW], bf16, tag="slab")

        # q^T / k^T stores [ch, tok]; v_store [tok-part, tok-tile, 160]
        qT = singles.tile([CH, TOK], bf16, tag="qT")
        kT = singles.tile([CH, TOK], bf16, tag="kT")
        # v_store cols: [v0: 0..63 | ones: 64 | zeros: 65..95 | v1: 96..159]
        # head0 lhsT = [:, t, 0:65]   -> acc rows 0-63 data, 64 sums
        # head1 lhsT = [:, t, 32:160] -> acc row 32 sums, rows 64-127 data
        v_store = singles.tile([128, TOK // 128, 160], p_dt, tag="vst")
        nc.vector.memset(v_store[:, :, 64:65], 1.0)
        nc.vector.memset(v_store[:, :, 65:96], 0.0)
        expb = singles.tile([128, 1], f32, tag="expb")
        nc.vector.memset(expb, EXP_BIAS)

        xt_v = xt.rearrange("(dt k) t -> k dt t", k=128)

        def proj_block(b, tb, emit_slab=False):
            ts = b * S + tb * TBP
            xt_sb = xt_pool.tile([128, DT, TBP], bf16, tag="xt", name=f"xt_{b}_{tb}")
            nc.sync.dma_start(out=xt_sb, in_=xt_v[:, :, ts : ts + TBP])
            if emit_slab:
                nc.sync.dma_start(out=slab_sb, in_=slab[:, :, :])
            for name in ("q", "k", "v"):
                pp = ps.tile([CH, TBP], f32, tag="pj", name=f"pj_{b}_{tb}_{name}")
                for dt in range(DT):
                    nc.tensor.matmul(
                        pp,
                        w_sb[name][:, dt],
                        xt_sb[:, dt],
                        start=(dt == 0),
                        stop=(dt == DT - 1),
                    )
                if name == "q":
                    nc.vector.tensor_scalar_add(
                        out=qT[:, ts : ts + TBP], in0=pp, scalar1=b_sb["q"]
                    )
                elif name == "k":
                    nc.vector.tensor_scalar_add(
                        out=kT[:, ts : ts + TBP], in0=pp, scalar1=b_sb["k"]
                    )
                else:
                    vt_sb = vt_pool.tile([CH, TBP], bf16, tag="vt", name=f"vt_{b}_{tb}")
                    nc.vector.tensor_scalar_add(out=vt_sb, in0=pp, scalar1=b_sb["v"])
                    for j in range(TBP // 128):
                        t_idx = (ts + j * 128) // 128
                        vps = ps.tile([128, 128], bf16, tag="pj", name=f"vps_{b}_{tb}_{j}")
                        nc.tensor.transpose(
                            vps, vt_sb[:, j * 128 : (j + 1) * 128], ident
                        )
                        nc.vector.tensor_copy(v_store[:, t_idx, 0:64], vps[:, 0:64])
                        nc.vector.tensor_copy(v_store[:, t_idx, 96:160], vps[:, 64:128])

        pend = []  # deferred small tensor-work units (oproj mm+dma)

        def drain_one():
            if pend:
                pend.pop(0)()

        def oproj_defer(b, qb, ao):
            base = b * S
            for i in range(SQ // 128):
                row = base + qb * SQ + i * 128
                for half in range(2):
                    def unit(i=i, half=half, row=row, ao=ao, b=b, qb=qb):
                        op = ps.tile(
                            [128, SQ], f32, tag="qk", name=f"op_{b}_{qb}_{i}_{half}"
                        )
                        nc.tensor.matmul(
                            op,
                            ao[:, i * 128 : (i + 1) * 128],
                            wo_sb[:, half * 512 : (half + 1) * 512],
                            start=True,
                            stop=True,
                        )
                        o_sb = out_pool.tile(
                            [128, SQ], f32, tag="o", name=f"o_{b}_{qb}_{i}_{half}"
                        )
                        nc.vector.tensor_copy(o_sb, op)
                        nc.sync.dma_start(
                            out=o_part[row : row + 128, half * 512 : (half + 1) * 512],
                            in_=o_sb,
                        )
                    if DEFER_OPROJ:
                        pend.append(unit)
                    else:
                        unit()

        def attn_v(b, qb, t2, p_sb, accA, accB):
            base = b * S
            for hh, acc, lo, w in ((0, accA, 0, 65), (1, accB, 32, 128)):
                t_idx = (base + 2 * t2 * SK) // 128
                if USE_FP8_ATTN:
                    nc.tensor.matmul(
                        acc[0:w, :],
                        v_store[:, t_idx : t_idx + 2, lo : lo + w],
                        p_sb[hh][:, :, :],
                        start=(t2 == 0),
                        stop=(t2 == NK2 - 1),
                        perf_mode=mybir.MatmulPerfMode.DoubleRow,
                    )
                else:
                    for i in (0, 1):
                        nc.tensor.matmul(
                            acc[0:w, :],
                            v_store[:, t_idx + i, lo : lo + w],
                            p_sb[hh][:, i, :],
                            start=(t2 == 0 and i == 0),
                            stop=(t2 == NK2 - 1 and i == 1),
                        )

        def attention(b):
            base = b * S
            for qb in range(NQB):
                q0 = base + qb * SQ
                accA = ps.tile([128, SQ], f32, tag="acc", name=f"accA_{b}_{qb}")
                accB = ps.tile([128, SQ], f32, tag="acc", name=f"accB_{b}_{qb}")
                prev = None
                for t2 in range(NK2):
                    # ---- QK for both heads, kt pair (paired row groups) ----
                    psh = [
                        ps.tile([128, 2, SQ], f32, tag="qk", name=f"qk_{b}_{qb}_{t2}_{hh}")
                        for hh in (0, 1)
                    ]
                    for i in (0, 1):
                        k0 = base + (2 * t2 + i) * SK
                        for hh in (0, 1):
                            nc.tensor.matmul(
                                psh[hh][:, i, :],
                                kT[hh * 64 : (hh + 1) * 64, k0 : k0 + SK],
                                qT[hh * 64 : (hh + 1) * 64, q0 : q0 + SQ],
                                start=True,
                                stop=(hh == 1),
                            )
                    # ---- head0 circular bias in psum: ident^T @ raw-bias slab ----
                    for i in (0, 1):
                        off = qb * SQ - (2 * t2 + i) * SK + U0
                        nc.tensor.matmul(
                            psh[0][:, i, :],
                            ident,
                            slab_sb[:, 0, off : off + SQ],
                            start=False,
                            stop=True,
                        )
                    drain_one()  # spread deferred oproj work into the loop
                    if prev is not None:
                        attn_v(b, qb, prev, prev_p, accA, accB)
                    # ---- head0: p0 = exp(scores+bias-2) directly from ACT ----
                    p_sb = [
                        p_pool.tile([128, 2, SQ], p_dt, tag="p", name=f"p_{b}_{qb}_{t2}_{hh}")
                        for hh in (0, 1)
                    ]
                    nc.scalar.activation(
                        out=p_sb[0],
                        in_=psh[0],
                        func=mybir.ActivationFunctionType.Exp,
                        bias=expb,
                    )
                    # ---- head1: e1 = exp(scores-2); p1 = e1 * exp-bias slab ----
                    e_sb = p_pool.tile([128, 2, SQ], bf16, tag="e", name=f"e_{b}_{qb}_{t2}")
                    nc.scalar.activation(
                        out=e_sb,
                        in_=psh[1],
                        func=mybir.ActivationFunctionType.Exp,
                        bias=expb,
                    )
                    for i in (0, 1):
                        off = qb * SQ - (2 * t2 + i) * SK + U0
                        eng = nc.vector if i == 0 else nc.gpsimd
                        eng.tensor_mul(
                            p_sb[1][:, i, :],
                            e_sb[:, i, :],
                            slab_sb[:, 1, off : off + SQ],
                        )
                    prev = t2
                    prev_p = p_sb
                attn_v(b, qb, prev, prev_p, accA, accB)
                # ---- normalize: 1/sum, broadcast, scale into attnout ----
                ao = ao_pool.tile([CH, SQ], bf16, tag="ao", name=f"ao_{b}_{qb}")
                for hh, acc, srow, dlo in ((0, accA, 64, 0), (1, accB, 32, 64)):
                    r = nrm_pool.tile([1, SQ], f32, tag="r", name=f"r_{b}_{qb}_{hh}")
                    rs = nrm_pool.tile([1, SQ], f32, tag="rs", name=f"rs_{b}_{qb}_{hh}")
                    nc.vector.tensor_copy(rs, acc[srow : srow + 1, :])
                    nc.vector.reciprocal_approx_fast(out=r, in_=rs)
                    rb = nrm_pool.tile([64, SQ], f32, tag="rb", name=f"rb_{b}_{qb}_{hh}")
                    nc.gpsimd.partition_broadcast(rb, r)
                    nc.vector.tensor_mul(ao[dlo : dlo + 64, :], acc[dlo : dlo + 64, :], rb)
                # ---- interleave next-batch projections under this attention
                if b == 0 and INTERLEAVE_PROJ:
                    proj_block(1, qb)
                oproj_defer(b, qb, ao)

        # ---- batch 0 projections (prologue) ----
        for tb in range(NTB):
            proj_block(0, tb, emit_slab=(tb == 0))
        if not INTERLEAVE_PROJ:
            for tb in range(NTB):
                proj_block(1, tb)
        attention(0)
        attention(1)
        while pend:
            drain_one()
    nc.compile()
    return nc


def _prep_inputs(x, wq, bq, wk, bk, wv, bv, wo, bo, rel_bias):
    """Host-side pack into per-core in_maps (all linear-DMA layouts)."""
    x = np.asarray(x, dtype=np.float32)
    rel_bias = np.asarray(rel_bias, dtype=np.float32)
    scale = 1.0 / math.sqrt(HD)

    xt = np.ascontiguousarray(x.reshape(TOK, D).T).astype(ml_dtypes.bfloat16)  # [D, TOK]

    # slab[p, u] = exp(rel_bias[(u - p - U0) % PERIOD, h])
    uu = np.arange(SLABW)[None, :]
    pp = np.arange(128)[:, None]
    slab_idx = (uu - pp - U0) % PERIOD  # [128, SLABW]

    in_maps = []
    for c in range(NCORES):
        sl = slice(c * CH, (c + 1) * CH)
        wq_c = (np.asarray(wq, np.float32)[:, sl] * scale).reshape(DT, 128, CH)
        wk_c = np.asarray(wk, np.float32)[:, sl].reshape(DT, 128, CH)
        wv_c = np.asarray(wv, np.float32)[:, sl].reshape(DT, 128, CH)
        slab_c = np.empty((128, 2, SLABW), dtype=ml_dtypes.bfloat16)
        # plane 0: raw bias for head0 (added to scores via identity matmul)
        slab_c[:, 0, :] = rel_bias[slab_idx, 2 * c]
        # plane 1: exp(bias) for head1 (multiplied into exp(scores) via TT)
        slab_c[:, 1, :] = np.exp(rel_bias[slab_idx, 2 * c + 1])
        in_maps.append(
            {
                "xt": xt,
                "wq": np.ascontiguousarray(wq_c.transpose(1, 0, 2)).astype(ml_dtypes.bfloat16),
                "wk": np.ascontiguousarray(wk_c.transpose(1, 0, 2)).astype(ml_dtypes.bfloat16),
                "wv": np.ascontiguousarray(wv_c.transpose(1, 0, 2)).astype(ml_dtypes.bfloat16),
                "wo": np.ascontiguousarray(np.asarray(wo, np.float32)[sl, :]).astype(ml_dtypes.bfloat16),
                "bq": (np.asarray(bq, np.float32)[sl] * scale).reshape(CH, 1),
                "bk": np.asarray(bk, np.float32)[sl].reshape(CH, 1),
                "bv": np.asarray(bv, np.float32)[sl].reshape(CH, 1),
                "slab": slab_c,
            }
        )
    return in_maps


def kernel(x, wq, bq, wk, bk, wv, bv, wo, bo, rel_bias, _trace=False):
    from concourse import bass_utils

    if "nc" not in _CACHE:
        _CACHE["nc"] = _build_nc()
    nc = _CACHE["nc"]

    in_maps = _prep_inputs(x, wq, bq, wk, bk, wv, bv, wo, bo, rel_bias)
    res = bass_utils.run_bass_kernel_spmd(
        nc, in_maps, core_ids=list(range(NCORES)), trace=_trace
    )
    _CACHE["last_result"] = res

    acc = np.zeros((TOK, D), dtype=np.float64)
    for r in res.results:
        acc += r["o_part"].astype(np.float64)
    acc += np.asarray(bo, np.float64)[None, :]
    return acc.reshape(B, S, D).astype(np.float32)


# revision 31
# speedup vs baseline: 1.1480x; 1.0671x over previous
"""Circular-relative-bias multi-head attention on 8 Trainium2 NeuronCores.

Sharding (Megatron MHA): 16 heads -> 2 heads per core. Each core computes
q/k/v projections for its 128 channels (2 heads x 64), full attention for
its heads over both batches, and a row-sharded output projection producing
a full-shape partial; the host sums the 8 partials and adds bo.

Pipeline layout (v2):
  - proj(b0) -> attn(b0) [proj(b1) + oproj(b0) interleaved] -> attn(b1)
    [oproj(b1) interleaved]; oproj matmul+DMA units are deferred and
    drained one per kt2 iteration of the NEXT qb so the tensor queue
    never stalls on an output DMA.
  - exp(rel_bias) enters as a per-head shifted slab [128, 3968]:
    slab[p, u] = exp(rel_bias[(u - p - 1920) % 4096]); the tile for
    (qb, kt) is the slice [*, off:off+512] with off = 512*qb - 128*kt
    + 1920.  1.94 MB DMA instead of a 7.3 MB precomputed tile table.
  - scores^T [sk, sq]; softmax sums come free from the attn@V matmul via
    ones-columns in v_store (head0 sums at accA row 64, head1 at accB
    row 32); reciprocal via reciprocal_approx_fast (~51 ULP, plenty).
  - attn@V pairs two consecutive key tiles [128, 2, 512]; with fp8
    (USE_FP8_ATTN) the pair contracts in one DoubleRow matmul at 2x rate.
"""

import math

import numpy as np
import ml_dtypes

B = 2
S = 2048
D = 1024
H = 16
HD = 64
PERIOD = 4096
NCORES = 8
CH = 128          # channels per core = 2 heads * 64
TOK = B * S       # 4096
DT = D // 128     # 8 k-tiles for the d contraction
TBP = 512         # token block for projections
NTB = S // TBP    # 4 per batch
SQ = 512          # sq block in attention
NQB = S // SQ     # 4 per batch
SK = 128          # sk tile
NKT = S // SK     # 16 per batch
NK2 = NKT // 2    # 8 kt-pairs
U0 = 1920         # slab shift so off >= 0
SLABW = 3 * SQ + (NKT - 1) * SK + U0 - 1920 + SQ  # 3968
USE_FP8_ATTN = False  # fp8 attn@V fails the 2e-2 gate (each of p/v alone ~2.2e-2)
EXP_BIAS = -2.0       # subtract from scores pre-exp (cancels in softmax)
DEFER_OPROJ = True    # spread oproj units into the next qb's kt2 loop
INTERLEAVE_PROJ = True  # emit b1 projections under b0 attention

_CACHE = {}


def _build_nc():
    import contextlib

    import concourse.tile as tile
    from concourse import bacc, mybir
    from concourse.masks import make_identity

    f32 = mybir.dt.float32
    bf16 = mybir.dt.bfloat16
    fp8 = mybir.dt.float8e4
    p_dt = fp8 if USE_FP8_ATTN else bf16

    nc = bacc.Bacc("TRN2")
    xt = nc.dram_tensor("xt", [D, TOK], bf16, kind="ExternalInput")
    wq = nc.dram_tensor("wq", [128, DT, CH], bf16, kind="ExternalInput")
    wk = nc.dram_tensor("wk", [128, DT, CH], bf16, kind="ExternalInput")
    wv = nc.dram_tensor("wv", [128, DT, CH], bf16, kind="ExternalInput")
    wo = nc.dram_tensor("wo", [CH, D], bf16, kind="ExternalInput")
    bq = nc.dram_tensor("bq", [CH, 1], f32, kind="ExternalInput")
    bk = nc.dram_tensor("bk", [CH, 1], f32, kind="ExternalInput")
    bv = nc.dram_tensor("bv", [CH, 1], f32, kind="ExternalInput")
    slab = nc.dram_tensor("slab", [128, 2, SLABW], bf16, kind="ExternalInput")
    o_part = nc.dram_tensor("o_part", [TOK, D], f32, kind="ExternalOutput")

    with tile.TileContext(nc) as tc, contextlib.ExitStack() as ctx:
        singles = ctx.enter_context(tc.tile_pool(name="singles", bufs=1))
        xt_pool = ctx.enter_context(tc.tile_pool(name="xtp", bufs=2))
        vt_pool = ctx.enter_context(tc.tile_pool(name="vtp", bufs=2))
        p_pool = ctx.enter_context(tc.tile_pool(name="pp", bufs=3))
        nrm_pool = ctx.enter_context(tc.tile_pool(name="nrm", bufs=2))
        ao_pool = ctx.enter_context(tc.tile_pool(name="ao", bufs=2))
        out_pool = ctx.enter_context(tc.tile_pool(name="outp", bufs=2))
        ps = ctx.enter_context(tc.tile_pool(name="ps", bufs=2, space="PSUM"))

        ident = singles.tile([128, 128], bf16)
        make_identity(nc, ident)

        w_sb = {}
        b_sb = {}
        for name, w_h, b_h in (("q", wq, bq), ("k", wk, bk), ("v", wv, bv)):
            w_sb[name] = singles.tile([128, DT, CH], bf16, tag=f"w{name}", name=f"w{name}_sb")
            nc.sync.dma_start(out=w_sb[name], in_=w_h[:, :, :])
            b_sb[name] = singles.tile([CH, 1], f32, tag=f"b{name}", name=f"b{name}_sb")
            nc.sync.dma_start(out=b_sb[name], in_=b_h[:, :])
        wo_sb = singles.tile([CH, D], bf16, tag="wo")
        nc.sync.dma_start(out=wo_sb, in_=wo[:, :])
        slab_sb = singles.tile([128, 2, SLABW], bf16, tag="slab")

        # q^T / k^T stores [ch, tok]; v_store [tok-part, tok-tile, 160]
        qT = singles.tile([CH, TOK], bf16, tag="qT")
        kT = singles.tile([CH, TOK], bf16, tag="kT")
        # v_store cols: [v0: 0..63 | ones: 64 | zeros: 65..95 | v1: 96..159]
        # head0 lhsT = [:, t, 0:65]   -> acc rows 0-63 data, 64 sums
        # head1 lhsT = [:, t, 32:160] -> acc row 32 sums, rows 64-127 data
        v_store = singles.tile([128, TOK // 128, 160], p_dt, tag="vst")
        nc.vector.memset(v_store[:, :, 64:65], 1.0)
        nc.vector.memset(v_store[:, :, 65:96], 0.0)
        expb = singles.tile([128, 1], f32, tag="expb")
        nc.vector.memset(expb, EXP_BIAS)

        xt_v = xt.rearrange("(dt k) t -> k dt t", k=128)

        pend = []  # deferred small tensor-work units (oproj / interleaved proj)

        def drain_one():
            n = 2 if len(pend) > 8 else 1
            for _ in range(min(n, len(pend))):
                pend.pop(0)()

        def proj_block(b, tb, emit_slab=False, defer=False):
            ts = b * S + tb * TBP
            xt_sb = xt_pool.tile([128, DT, TBP], bf16, tag="xt", name=f"xt_{b}_{tb}")
            nc.sync.dma_start(out=xt_sb, in_=xt_v[:, :, ts : ts + TBP])
            if emit_slab:
                nc.sync.dma_start(out=slab_sb, in_=slab[:, :, :])

            def name_unit(name):
                pp = ps.tile([CH, TBP], f32, tag="pj", name=f"pj_{b}_{tb}_{name}")
                for dt in range(DT):
                    nc.tensor.matmul(
                        pp,
                        w_sb[name][:, dt],
                        xt_sb[:, dt],
                        start=(dt == 0),
                        stop=(dt == DT - 1),
                    )
                if name == "q":
                    nc.vector.tensor_scalar_add(
                        out=qT[:, ts : ts + TBP], in0=pp, scalar1=b_sb["q"]
                    )
                elif name == "k":
                    nc.vector.tensor_scalar_add(
                        out=kT[:, ts : ts + TBP], in0=pp, scalar1=b_sb["k"]
                    )
                else:
                    vt_sb = vt_pool.tile([CH, TBP], bf16, tag="vt", name=f"vt_{b}_{tb}")
                    nc.vector.tensor_scalar_add(out=vt_sb, in0=pp, scalar1=b_sb["v"])
                    _CACHE_VT[(b, tb)] = vt_sb

            def vps_unit():
                vt_sb = _CACHE_VT.pop((b, tb))
                for j in range(TBP // 128):
                    t_idx = (ts + j * 128) // 128
                    vps = ps.tile([128, 128], bf16, tag="pj", name=f"vps_{b}_{tb}_{j}")
                    nc.tensor.transpose(vps, vt_sb[:, j * 128 : (j + 1) * 128], ident)
                    nc.vector.tensor_copy(v_store[:, t_idx, 0:64], vps[:, 0:64])
                    nc.vector.tensor_copy(v_store[:, t_idx, 96:160], vps[:, 64:128])

            units = [lambda n=n: name_unit(n) for n in ("q", "k", "v")] + [vps_unit]
            if defer:
                pend.extend(units)
            else:
                for u in units:
                    u()

        _CACHE_VT = {}

        def oproj_defer(b, qb, ao):
            base = b * S
            for i in range(SQ // 128):
                row = base + qb * SQ + i * 128
                for half in range(2):
                    def unit(i=i, half=half, row=row, ao=ao, b=b, qb=qb):
                        op = ps.tile(
                            [128, SQ], f32, tag="qk", name=f"op_{b}_{qb}_{i}_{half}"
                        )
                        nc.tensor.matmul(
                            op,
                            ao[:, i * 128 : (i + 1) * 128],
                            wo_sb[:, half * 512 : (half + 1) * 512],
                            start=True,
                            stop=True,
                        )
                        o_sb = out_pool.tile(
                            [128, SQ], f32, tag="o", name=f"o_{b}_{qb}_{i}_{half}"
                        )
                        nc.vector.tensor_copy(o_sb, op)
                        nc.sync.dma_start(
                            out=o_part[row : row + 128, half * 512 : (half + 1) * 512],
                            in_=o_sb,
                        )
                    if DEFER_OPROJ:
                        pend.append(unit)
                    else:
                        unit()

        def attn_v(b, qb, t2, p_sb, accA, accB):
            base = b * S
            for hh, acc, lo, w in ((0, accA, 0, 65), (1, accB, 32, 128)):
                t_idx = (base + 2 * t2 * SK) // 128
                if USE_FP8_ATTN:
                    nc.tensor.matmul(
                        acc[0:w, :],
                        v_store[:, t_idx : t_idx + 2, lo : lo + w],
                        p_sb[hh][:, :, :],
                        start=(t2 == 0),
                        stop=(t2 == NK2 - 1),
                        perf_mode=mybir.MatmulPerfMode.DoubleRow,
                    )
                else:
                    for i in (0, 1):
                        nc.tensor.matmul(
                            acc[0:w, :],
                            v_store[:, t_idx + i, lo : lo + w],
                            p_sb[hh][:, i, :],
                            start=(t2 == 0 and i == 0),
                            stop=(t2 == NK2 - 1 and i == 1),
                        )

        def attention(b):
            base = b * S
            for qb in range(NQB):
                q0 = base + qb * SQ
                accA = ps.tile([128, SQ], f32, tag="acc", name=f"accA_{b}_{qb}")
                accB = ps.tile([128, SQ], f32, tag="acc", name=f"accB_{b}_{qb}")
                hist = []  # (t2, p_sb) pending attn_v emissions, depth-2 pipeline
                for t2 in range(NK2):
                    # ---- QK for both heads, kt pair (paired row groups) ----
                    psh = [
                        ps.tile([128, 2, SQ], f32, tag="qk", name=f"qk_{b}_{qb}_{t2}_{hh}")
                        for hh in (0, 1)
                    ]
                    for i in (0, 1):
                        k0 = base + (2 * t2 + i) * SK
                        for hh in (0, 1):
                            nc.tensor.matmul(
                                psh[hh][:, i, :],
                                kT[hh * 64 : (hh + 1) * 64, k0 : k0 + SK],
                                qT[hh * 64 : (hh + 1) * 64, q0 : q0 + SQ],
                                start=True,
                                stop=(hh == 1),
                            )
                    # ---- head0 circular bias in psum: ident^T @ raw-bias slab ----
                    for i in (0, 1):
                        off = qb * SQ - (2 * t2 + i) * SK + U0
                        nc.tensor.matmul(
                            psh[0][:, i, :],
                            ident,
                            slab_sb[:, 0, off : off + SQ],
                            start=False,
                            stop=True,
                        )
                    drain_one()  # spread deferred oproj/proj work into the loop
                    if len(hist) >= 2:
                        pt2, pp_sb = hist.pop(0)
                        attn_v(b, qb, pt2, pp_sb, accA, accB)
                    # ---- head0: p0 = exp(scores+bias-2) directly from ACT ----
                    p_sb = [
                        p_pool.tile(
                            [128, 2, SQ], p_dt, tag="p", bufs=6,
                            name=f"p_{b}_{qb}_{t2}_{hh}",
                        )
                        for hh in (0, 1)
                    ]
                    nc.scalar.activation(
                        out=p_sb[0],
                        in_=psh[0],
                        func=mybir.ActivationFunctionType.Exp,
                        bias=expb,
                    )
                    # ---- head1: e1 = exp(scores-2); p1 = e1 * exp-bias slab ----
                    e_sb = p_pool.tile([128, 2, SQ], bf16, tag="e", name=f"e_{b}_{qb}_{t2}")
                    nc.scalar.activation(
                        out=e_sb,
                        in_=psh[1],
                        func=mybir.ActivationFunctionType.Exp,
                        bias=expb,
                    )
                    for i in (0, 1):
                        off = qb * SQ - (2 * t2 + i) * SK + U0
                        eng = nc.vector if i == 0 else nc.gpsimd
                        eng.tensor_mul(
                            p_sb[1][:, i, :],
                            e_sb[:, i, :],
                            slab_sb[:, 1, off : off + SQ],
                        )
                    hist.append((t2, p_sb))
                for pt2, pp_sb in hist:
                    attn_v(b, qb, pt2, pp_sb, accA, accB)
                # ---- normalize: 1/sum, broadcast, scale into attnout ----
                ao = ao_pool.tile([CH, SQ], bf16, tag="ao", name=f"ao_{b}_{qb}")
                for hh, acc, srow, dlo in ((0, accA, 64, 0), (1, accB, 32, 64)):
                    r = nrm_pool.tile([1, SQ], f32, tag="r", name=f"r_{b}_{qb}_{hh}")
                    rs = nrm_pool.tile([1, SQ], f32, tag="rs", name=f"rs_{b}_{qb}_{hh}")
                    nc.scalar.copy(rs, acc[srow : srow + 1, :])
                    nc.vector.reciprocal_approx_fast(out=r, in_=rs)
                    rb = nrm_pool.tile([64, SQ], f32, tag="rb", name=f"rb_{b}_{qb}_{hh}")
                    nc.gpsimd.partition_broadcast(rb, r)
                    nc.vector.tensor_mul(ao[dlo : dlo + 64, :], acc[dlo : dlo + 64, :], rb)
                # ---- interleave next-batch projections under this attention
                if b == 0 and INTERLEAVE_PROJ:
                    proj_block(1, qb, defer=True)
                oproj_defer(b, qb, ao)

        # ---- batch 0 projections (prologue) ----
        for tb in range(NTB):
            proj_block(0, tb, emit_slab=(tb == 0))
        if not INTERLEAVE_PROJ:
            for tb in range(NTB):
                proj_block(1, tb)
        attention(0)
        attention(1)
        while pend:
            drain_one()
    nc.compile()
    return nc


def _prep_inputs(x, wq, bq, wk, bk, wv, bv, wo, bo, rel_bias):
    """Host-side pack into per-core in_maps (all linear-DMA layouts)."""
    x = np.asarray(x, dtype=np.float32)
    rel_bias = np.asarray(rel_bias, dtype=np.float32)
    scale = 1.0 / math.sqrt(HD)

    xt = np.ascontiguousarray(x.reshape(TOK, D).T).astype(ml_dtypes.bfloat16)  # [D, TOK]

    # slab[p, u] = exp(rel_bias[(u - p - U0) % PERIOD, h])
    uu = np.arange(SLABW)[None, :]
    pp = np.arange(128)[:, None]
    slab_idx = (uu - pp - U0) % PERIOD  # [128, SLABW]

    in_maps = []
    for c in range(NCORES):
        sl = slice(c * CH, (c + 1) * CH)
        wq_c = (np.asarray(wq, np.float32)[:, sl] * scale).reshape(DT, 128, CH)
        wk_c = np.asarray(wk, np.float32)[:, sl].reshape(DT, 128, CH)
        wv_c = np.asarray(wv, np.float32)[:, sl].reshape(DT, 128, CH)
        slab_c = np.empty((128, 2, SLABW), dtype=ml_dtypes.bfloat16)
        # plane 0: raw bias for head0 (added to scores via identity matmul)
        slab_c[:, 0, :] = rel_bias[slab_idx, 2 * c]
        # plane 1: exp(bias) for head1 (multiplied into exp(scores) via TT)
        slab_c[:, 1, :] = np.exp(rel_bias[slab_idx, 2 * c + 1])
        in_maps.append(
            {
                "xt": xt,
                "wq": np.ascontiguousarray(wq_c.transpose(1, 0, 2)).astype(ml_dtypes.bfloat16),
                "wk": np.ascontiguousarray(wk_c.transpose(1, 0, 2)).astype(ml_dtypes.bfloat16),
                "wv": np.ascontiguousarray(wv_c.transpose(1, 0, 2)).astype(ml_dtypes.bfloat16),
                "wo": np.ascontiguousarray(np.asarray(wo, np.float32)[sl, :]).astype(ml_dtypes.bfloat16),
                "bq": (np.asarray(bq, np.float32)[sl] * scale).reshape(CH, 1),
                "bk": np.asarray(bk, np.float32)[sl].reshape(CH, 1),
                "bv": np.asarray(bv, np.float32)[sl].reshape(CH, 1),
                "slab": slab_c,
            }
        )
    return in_maps


def kernel(x, wq, bq, wk, bk, wv, bv, wo, bo, rel_bias, _trace=False):
    from concourse import bass_utils

    if "nc" not in _CACHE:
        _CACHE["nc"] = _build_nc()
    nc = _CACHE["nc"]

    in_maps = _prep_inputs(x, wq, bq, wk, bk, wv, bv, wo, bo, rel_bias)
    res = bass_utils.run_bass_kernel_spmd(
        nc, in_maps, core_ids=list(range(NCORES)), trace=_trace
    )
    _CACHE["last_result"] = res

    acc = np.zeros((TOK, D), dtype=np.float64)
    for r in res.results:
        acc += r["o_part"].astype(np.float64)
    acc += np.asarray(bo, np.float64)[None, :]
    return acc.reshape(B, S, D).astype(np.float32)


# revision 34
# speedup vs baseline: 1.1875x; 1.0344x over previous
"""Circular-relative-bias multi-head attention on 8 Trainium2 NeuronCores.

Sharding (Megatron MHA): 16 heads -> 2 heads per core. Each core computes
q/k/v projections for its 128 channels (2 heads x 64), full attention for
its heads over both batches, and a row-sharded output projection producing
a full-shape partial; the host sums the 8 partials and adds bo.

Pipeline layout (v2):
  - proj(b0) -> attn(b0) [proj(b1) + oproj(b0) interleaved] -> attn(b1)
    [oproj(b1) interleaved]; oproj matmul+DMA units are deferred and
    drained one per kt2 iteration of the NEXT qb so the tensor queue
    never stalls on an output DMA.
  - exp(rel_bias) enters as a per-head shifted slab [128, 3968]:
    slab[p, u] = exp(rel_bias[(u - p - 1920) % 4096]); the tile for
    (qb, kt) is the slice [*, off:off+512] with off = 512*qb - 128*kt
    + 1920.  1.94 MB DMA instead of a 7.3 MB precomputed tile table.
  - scores^T [sk, sq]; softmax sums come free from the attn@V matmul via
    ones-columns in v_store (head0 sums at accA row 64, head1 at accB
    row 32); reciprocal via reciprocal_approx_fast (~51 ULP, plenty).
  - attn@V pairs two consecutive key tiles [128, 2, 512]; with fp8
    (USE_FP8_ATTN) the pair contracts in one DoubleRow matmul at 2x rate.
"""

import math

import numpy as np
import ml_dtypes

B = 2
S = 2048
D = 1024
H = 16
HD = 64
PERIOD = 4096
NCORES = 8
CH = 128          # channels per core = 2 heads * 64
TOK = B * S       # 4096
DT = D // 128     # 8 k-tiles for the d contraction
TBP = 256         # token block for projections
NTB = S // TBP    # 8 per batch
SQ = 512          # sq block in attention
NQB = S // SQ     # 4 per batch
SK = 128          # sk tile
NKT = S // SK     # 16 per batch
NK2 = NKT // 2    # 8 kt-pairs
U0 = 1920         # slab shift so off >= 0
SLABW = 3 * SQ + (NKT - 1) * SK + U0 - 1920 + SQ  # 3968
USE_FP8_ATTN = False  # fp8 attn@V fails the 2e-2 gate (each of p/v alone ~2.2e-2)
EXP_BIAS = -2.0       # subtract from scores pre-exp (cancels in softmax)
DEFER_OPROJ = True    # spread oproj units into the next qb's kt2 loop
INTERLEAVE_PROJ = True  # emit b1 projections under b0 attention

_CACHE = {}


def _build_nc():
    import contextlib

    import concourse.tile as tile
    from concourse import bacc, mybir
    from concourse.masks import make_identity

    f32 = mybir.dt.float32
    bf16 = mybir.dt.bfloat16
    fp8 = mybir.dt.float8e4
    p_dt = fp8 if USE_FP8_ATTN else bf16

    nc = bacc.Bacc("TRN2")
    xt = nc.dram_tensor("xt", [D, TOK], bf16, kind="ExternalInput")
    wq = nc.dram_tensor("wq", [128, DT, CH], bf16, kind="ExternalInput")
    wk = nc.dram_tensor("wk", [128, DT, CH], bf16, kind="ExternalInput")
    wv = nc.dram_tensor("wv", [128, DT, CH], bf16, kind="ExternalInput")
    wo = nc.dram_tensor("wo", [CH, D], bf16, kind="ExternalInput")
    bq = nc.dram_tensor("bq", [CH, 1], f32, kind="ExternalInput")
    bk = nc.dram_tensor("bk", [CH, 1], f32, kind="ExternalInput")
    bv = nc.dram_tensor("bv", [CH, 1], f32, kind="ExternalInput")
    slab = nc.dram_tensor("slab", [128, 2, SLABW], bf16, kind="ExternalInput")
    o_part = nc.dram_tensor("o_part", [TOK, D], f32, kind="ExternalOutput")

    with tile.TileContext(nc) as tc, contextlib.ExitStack() as ctx:
        singles = ctx.enter_context(tc.tile_pool(name="singles", bufs=1))
        xt_pool = ctx.enter_context(tc.tile_pool(name="xtp", bufs=2))
        vt_pool = ctx.enter_context(tc.tile_pool(name="vtp", bufs=2))
        p_pool = ctx.enter_context(tc.tile_pool(name="pp", bufs=3))
        nrm_pool = ctx.enter_context(tc.tile_pool(name="nrm", bufs=2))
        ao_pool = ctx.enter_context(tc.tile_pool(name="ao", bufs=2))
        out_pool = ctx.enter_context(tc.tile_pool(name="outp", bufs=2))
        ps = ctx.enter_context(tc.tile_pool(name="ps", bufs=2, space="PSUM"))

        ident = singles.tile([128, 128], bf16)
        make_identity(nc, ident)

        w_sb = {}
        b_sb = {}
        for name, w_h, b_h in (("q", wq, bq), ("k", wk, bk), ("v", wv, bv)):
            w_sb[name] = singles.tile([128, DT, CH], bf16, tag=f"w{name}", name=f"w{name}_sb")
            nc.sync.dma_start(out=w_sb[name], in_=w_h[:, :, :])
            b_sb[name] = singles.tile([CH, 1], f32, tag=f"b{name}", name=f"b{name}_sb")
            nc.sync.dma_start(out=b_sb[name], in_=b_h[:, :])
        wo_sb = singles.tile([CH, D], bf16, tag="wo")
        nc.sync.dma_start(out=wo_sb, in_=wo[:, :])
        slab_sb = singles.tile([128, 2, SLABW], bf16, tag="slab")

        # q^T / k^T stores [ch, tok]; v_store [tok-part, tok-tile, 160]
        qT = singles.tile([CH, TOK], bf16, tag="qT")
        kT = singles.tile([CH, TOK], bf16, tag="kT")
        # v_store cols: [v0: 0..63 | ones: 64 | zeros: 65..95 | v1: 96..159]
        # head0 lhsT = [:, t, 0:65]   -> acc rows 0-63 data, 64 sums
        # head1 lhsT = [:, t, 32:160] -> acc row 32 sums, rows 64-127 data
        v_store = singles.tile([128, TOK // 128, 160], p_dt, tag="vst")
        nc.vector.memset(v_store[:, :, 64:65], 1.0)
        nc.vector.memset(v_store[:, :, 65:96], 0.0)
        expb = singles.tile([128, 1], f32, tag="expb")
        nc.vector.memset(expb, EXP_BIAS)

        xt_v = xt.rearrange("(dt k) t -> k dt t", k=128)

        pend = []  # deferred small tensor-work units (oproj / interleaved proj)

        def drain_one():
            n = 2 if len(pend) > 8 else 1
            for _ in range(min(n, len(pend))):
                pend.pop(0)()

        def proj_block(b, tb, emit_slab=False, defer=False):
            ts = b * S + tb * TBP
            xt_sb = xt_pool.tile([128, DT, TBP], bf16, tag="xt", name=f"xt_{b}_{tb}")
            nc.sync.dma_start(out=xt_sb, in_=xt_v[:, :, ts : ts + TBP])
            if emit_slab:
                nc.sync.dma_start(out=slab_sb, in_=slab[:, :, :])

            def name_unit(name):
                pp = ps.tile([CH, TBP], f32, tag="pj", bufs=1, name=f"pj_{b}_{tb}_{name}")
                for dt in range(DT):
                    nc.tensor.matmul(
                        pp,
                        w_sb[name][:, dt],
                        xt_sb[:, dt],
                        start=(dt == 0),
                        stop=(dt == DT - 1),
                    )
                if name == "q":
                    nc.vector.tensor_scalar_add(
                        out=qT[:, ts : ts + TBP], in0=pp, scalar1=b_sb["q"]
                    )
                elif name == "k":
                    nc.vector.tensor_scalar_add(
                        out=kT[:, ts : ts + TBP], in0=pp, scalar1=b_sb["k"]
                    )
                else:
                    vt_sb = vt_pool.tile([CH, TBP], bf16, tag="vt", name=f"vt_{b}_{tb}")
                    nc.vector.tensor_scalar_add(out=vt_sb, in0=pp, scalar1=b_sb["v"])
                    _CACHE_VT[(b, tb)] = vt_sb

            def vps_unit():
                vt_sb = _CACHE_VT.pop((b, tb))
                for j in range(TBP // 128):
                    t_idx = (ts + j * 128) // 128
                    vps = ps.tile([128, 128], bf16, tag="pj", bufs=1, name=f"vps_{b}_{tb}_{j}")
                    nc.tensor.transpose(vps, vt_sb[:, j * 128 : (j + 1) * 128], ident)
                    nc.vector.tensor_copy(v_store[:, t_idx, 0:64], vps[:, 0:64])
                    nc.vector.tensor_copy(v_store[:, t_idx, 96:160], vps[:, 64:128])

            units = [lambda n=n: name_unit(n) for n in ("q", "k", "v")] + [vps_unit]
            if defer:
                pend.extend(units)
            else:
                for u in units:
                    u()

        _CACHE_VT = {}

        def oproj_defer(b, qb, ao):
            base = b * S
            for i in range(SQ // 128):
                row = base + qb * SQ + i * 128
                for half in range(2):
                    def unit(i=i, half=half, row=row, ao=ao, b=b, qb=qb):
                        op = ps.tile(
                            [128, SQ], f32, tag="op", bufs=1,
                            name=f"op_{b}_{qb}_{i}_{half}",
                        )
                        nc.tensor.matmul(
                            op,
                            ao[:, i * 128 : (i + 1) * 128],
                            wo_sb[:, half * 512 : (half + 1) * 512],
                            start=True,
                            stop=True,
                        )
                        o_sb = out_pool.tile(
                            [128, SQ], f32, tag="o", name=f"o_{b}_{qb}_{i}_{half}"
                        )
                        nc.vector.tensor_copy(o_sb, op)
                        nc.sync.dma_start(
                            out=o_part[row : row + 128, half * 512 : (half + 1) * 512],
                            in_=o_sb,
                        )
                    if DEFER_OPROJ:
                        pend.append(unit)
                    else:
                        unit()

        def attn_v(b, qb, t2, p_sb, accA, accB):
            base = b * S
            for hh, acc, lo, w in ((0, accA, 0, 65), (1, accB, 32, 128)):
                t_idx = (base + 2 * t2 * SK) // 128
                if USE_FP8_ATTN:
                    nc.tensor.matmul(
                        acc[0:w, :],
                        v_store[:, t_idx : t_idx + 2, lo : lo + w],
                        p_sb[hh][:, :, :],
                        start=(t2 == 0),
                        stop=(t2 == NK2 - 1),
                        perf_mode=mybir.MatmulPerfMode.DoubleRow,
                    )
                else:
                    for i in (0, 1):
                        nc.tensor.matmul(
                            acc[0:w, :],
                            v_store[:, t_idx + i, lo : lo + w],
                            p_sb[hh][:, i, :],
                            start=(t2 == 0 and i == 0),
                            stop=(t2 == NK2 - 1 and i == 1),
                        )

        def attention(b):
            base = b * S
            for qb in range(NQB):
                q0 = base + qb * SQ
                accA = ps.tile([128, SQ], f32, tag="acc", name=f"accA_{b}_{qb}")
                accB = ps.tile([128, SQ], f32, tag="acc", name=f"accB_{b}_{qb}")
                hist = []  # (t2, p_sb) pending attn_v emissions, depth-2 pipeline
                for t2 in range(NK2):
                    # ---- QK for both heads, kt pair (paired row groups) ----
                    psh = [
                        ps.tile([128, 2, SQ], f32, tag="qk", name=f"qk_{b}_{qb}_{t2}_{hh}")
                        for hh in (0, 1)
                    ]
                    for i in (0, 1):
                        k0 = base + (2 * t2 + i) * SK
                        for hh in (0, 1):
                            nc.tensor.matmul(
                                psh[hh][:, i, :],
                                kT[hh * 64 : (hh + 1) * 64, k0 : k0 + SK],
                                qT[hh * 64 : (hh + 1) * 64, q0 : q0 + SQ],
                                start=True,
                                stop=(hh == 1),
                            )
                    # ---- head0 circular bias in psum: ident^T @ raw-bias slab ----
                    for i in (0, 1):
                        off = qb * SQ - (2 * t2 + i) * SK + U0
                        nc.tensor.matmul(
                            psh[0][:, i, :],
                            ident,
                            slab_sb[:, 0, off : off + SQ],
                            start=False,
                            stop=True,
                        )
                    drain_one()  # spread deferred oproj/proj work into the loop
                    if len(hist) >= 2:
                        pt2, pp_sb = hist.pop(0)
                        attn_v(b, qb, pt2, pp_sb, accA, accB)
                    # ---- head0: p0 = exp(scores+bias-2) directly from ACT ----
                    p_sb = [
                        p_pool.tile(
                            [128, 2, SQ], p_dt, tag="p", bufs=6,
                            name=f"p_{b}_{qb}_{t2}_{hh}",
                        )
                        for hh in (0, 1)
                    ]
                    nc.scalar.activation(
                        out=p_sb[0],
                        in_=psh[0],
                        func=mybir.ActivationFunctionType.Exp,
                        bias=expb,
                    )
                    # ---- head1: e1 = exp(scores-2); p1 = e1 * exp-bias slab ----
                    e_sb = p_pool.tile([128, 2, SQ], bf16, tag="e", name=f"e_{b}_{qb}_{t2}")
                    nc.scalar.activation(
                        out=e_sb,
                        in_=psh[1],
                        func=mybir.ActivationFunctionType.Exp,
                        bias=expb,
                    )
                    for i in (0, 1):
                        off = qb * SQ - (2 * t2 + i) * SK + U0
                        eng = nc.vector if i == 0 else nc.gpsimd
                        eng.tensor_mul(
                            p_sb[1][:, i, :],
                            e_sb[:, i, :],
                            slab_sb[:, 1, off : off + SQ],
                        )
                    hist.append((t2, p_sb))
                for pt2, pp_sb in hist:
                    attn_v(b, qb, pt2, pp_sb, accA, accB)
                # ---- normalize: 1/sum, broadcast, scale into attnout ----
                ao = ao_pool.tile([CH, SQ], bf16, tag="ao", name=f"ao_{b}_{qb}")
                for hh, acc, srow, dlo in ((0, accA, 64, 0), (1, accB, 32, 64)):
                    r = nrm_pool.tile([1, SQ], f32, tag="r", name=f"r_{b}_{qb}_{hh}")
                    rs = nrm_pool.tile([1, SQ], f32, tag="rs", name=f"rs_{b}_{qb}_{hh}")
                    nc.scalar.copy(rs, acc[srow : srow + 1, :])
                    nc.vector.reciprocal_approx_fast(out=r, in_=rs)
                    rb = nrm_pool.tile([64, SQ], f32, tag="rb", name=f"rb_{b}_{qb}_{hh}")
                    nc.gpsimd.partition_broadcast(rb, r)
                    nc.vector.tensor_mul(ao[dlo : dlo + 64, :], acc[dlo : dlo + 64, :], rb)
                # ---- interleave next-batch projections under this attention
                if b == 0 and INTERLEAVE_PROJ:
                    for tb in range(qb * NTB // NQB, (qb + 1) * NTB // NQB):
                        proj_block(1, tb, defer=True)
                oproj_defer(b, qb, ao)

        # ---- batch 0 projections (prologue) ----
        for tb in range(NTB):
            proj_block(0, tb, emit_slab=(tb == 0))
        if not INTERLEAVE_PROJ:
            for tb in range(NTB):
                proj_block(1, tb)
        attention(0)
        attention(1)
        while pend:
            drain_one()
    nc.compile()
    return nc


def _prep_inputs(x, wq, bq, wk, bk, wv, bv, wo, bo, rel_bias):
    """Host-side pack into per-core in_maps (all linear-DMA layouts)."""
    x = np.asarray(x, dtype=np.float32)
    rel_bias = np.asarray(rel_bias, dtype=np.float32)
    scale = 1.0 / math.sqrt(HD)

    xt = np.ascontiguousarray(x.reshape(TOK, D).T).astype(ml_dtypes.bfloat16)  # [D, TOK]

    # slab[p, u] = exp(rel_bias[(u - p - U0) % PERIOD, h])
    uu = np.arange(SLABW)[None, :]
    pp = np.arange(128)[:, None]
    slab_idx = (uu - pp - U0) % PERIOD  # [128, SLABW]

    in_maps = []
    for c in range(NCORES):
        sl = slice(c * CH, (c + 1) * CH)
        wq_c = (np.asarray(wq, np.float32)[:, sl] * scale).reshape(DT, 128, CH)
        wk_c = np.asarray(wk, np.float32)[:, sl].reshape(DT, 128, CH)
        wv_c = np.asarray(wv, np.float32)[:, sl].reshape(DT, 128, CH)
        slab_c = np.empty((128, 2, SLABW), dtype=ml_dtypes.bfloat16)
        # plane 0: raw bias for head0 (added to scores via identity matmul)
        slab_c[:, 0, :] = rel_bias[slab_idx, 2 * c]
        # plane 1: exp(bias) for head1 (multiplied into exp(scores) via TT)
        slab_c[:, 1, :] = np.exp(rel_bias[slab_idx, 2 * c + 1])
        in_maps.append(
            {
                "xt": xt,
                "wq": np.ascontiguousarray(wq_c.transpose(1, 0, 2)).astype(ml_dtypes.bfloat16),
                "wk": np.ascontiguousarray(wk_c.transpose(1, 0, 2)).astype(ml_dtypes.bfloat16),
                "wv": np.ascontiguousarray(wv_c.transpose(1, 0, 2)).astype(ml_dtypes.bfloat16),
                "wo": np.ascontiguousarray(np.asarray(wo, np.float32)[sl, :]).astype(ml_dtypes.bfloat16),
                "bq": (np.asarray(bq, np.float32)[sl] * scale).reshape(CH, 1),
                "bk": np.asarray(bk, np.float32)[sl].reshape(CH, 1),
                "bv": np.asarray(bv, np.float32)[sl].reshape(CH, 1),
                "slab": slab_c,
            }
        )
    return in_maps


def kernel(x, wq, bq, wk, bk, wv, bv, wo, bo, rel_bias, _trace=False):
    from concourse import bass_utils

    if "nc" not in _CACHE:
        _CACHE["nc"] = _build_nc()
    nc = _CACHE["nc"]

    in_maps = _prep_inputs(x, wq, bq, wk, bk, wv, bv, wo, bo, rel_bias)
    res = bass_utils.run_bass_kernel_spmd(
        nc, in_maps, core_ids=list(range(NCORES)), trace=_trace
    )
    _CACHE["last_result"] = res

    acc = np.zeros((TOK, D), dtype=np.float64)
    for r in res.results:
        acc += r["o_part"].astype(np.float64)
    acc += np.asarray(bo, np.float64)[None, :]
    return acc.reshape(B, S, D).astype(np.float32)
